# revision 1
# baseline (speedup 1.0000x reference)
"""Trainium2 Bass kernel for nn_Adaptive_dilatedConv (dense_cnn).

Reference computation (per image):
  logits = einsum('bchw,kc->bkhw', x, attn_w) + attn_b        # [B,3,H,W]
  attn   = softmax(logits, axis=1)
  convs_k = depthwise3x3(x, dw_w[k], dilation d_k) + dw_b[k]  # [B,C,H,W] x3
  fused  = sum_k convs_k * (attn_k + 1)
  out    = einsum('bchw,oc->bohw', fused, out_w) + out_b

Distribution: pure data parallelism over batch (16 images over 8 cores,
2 images per core); all parameters replicated.

Per-core schedule (channels on partitions, spatial flattened on free dim),
engineered so DVE/ACT/PE all stay busy concurrently:
  - x is loaded bf16 into persistent zero-padded [128, 74, 74] frames per
    channel-chunk, twice (an element-shifted "odd" copy keeps every
    dilated-tap view 4-byte aligned so the DVE runs its 2x/4x packed modes).
  - attention (both images up front): per-hw-tile matmuls with lhsT = x give
    logits transposed [hw, 3]; softmax runs entirely in that layout (tiny
    [128, 96] ops, exp on ScalarE, fast-reciprocal on DVE, exp(attn_b)
    folded in as host immediates), then one TensorE transpose + a
    DRAM-bounced partition-broadcast DMA produce (1 + attn_k) as [128, hw]
    bf16 tensors.
  - depthwise branches d=1 and d=5 on the elementwise engines: per tap a
    product (DVE tensor_scalar at 4x bf16, or ScalarE activation-Copy with
    per-partition scale for 6 of 9 taps) and a DVE tensor_tensor add (2x),
    interleaved so the product pool recycles; (1 + attn_k) applied in place.
  - depthwise branch d=2 rides the TensorEngine: host precomputes
    gwt[t] = out_w ⊙ dw_w[1,:,t] so P_g = sum_t gwt[t] @ x_shift_t
    accumulates the branch's conv AND its 1x1 out-conv in PSUM (18 matmuls
    per 512-column tile, streamed during the tap phase); one DVE
    scalar_tensor_tensor forms t_g = (P_g + out_w@dw_b[1]) * (1+attn_1).
  - out conv: P_a = sum OW @ fused (4 matmuls/tile) and a final
    scalar_tensor_tensor (P_a + out_b) + t_g writes f32 straight from PSUM.
  - DMAs spread across the SP/ACT HWDGE and gpsimd SWDGE lanes; double
    buffering on chunk-0 accumulators decouples consecutive images.
"""

import os
import sys

import numpy as np

sys.path.insert(0, "/opt/trn_rl_repo")

import concourse.bass as bass  # noqa: E402
import concourse.bacc as bacc  # noqa: E402
import concourse.mybir as mybir  # noqa: E402
import concourse.tile as tile  # noqa: E402
from concourse.masks import make_identity  # noqa: E402

F32 = mybir.dt.float32
BF16 = mybir.dt.bfloat16

N_CORES = 8
B, C, H, W = 16, 256, 64, 64
PB = B // N_CORES  # images per core
DILATIONS = (1, 2, 5)
PAD = 5
WP = W + 2 * PAD  # 74
HW = H * W  # 4096
NB = 8  # 512-column blocks per image for the out conv
NBW = HW // NB // W  # rows per block = 8

# taps per branch handled by the Scalar engine instead of the DVE
ACT_TAPS = (1, 2, 3, 4, 5, 7, 8)

AluOp = mybir.AluOpType
ActFn = mybir.ActivationFunctionType


def build_bass(u_vals, reps=1):
    """Build the single-core Bass graph. u_vals = exp(attn_b) host floats.

    reps > 1 unrolls the whole per-core computation multiple times inside
    one NEFF — used only for wall-clock-difference timing in test.py."""
    nc = bacc.Bacc()

    xb = nc.declare_dram_parameter("xb", [PB, C, H, W], BF16, isOutput=False)
    dwp = nc.declare_dram_parameter("dwp", [128, 2, 30], F32, isOutput=False)
    awp = nc.declare_dram_parameter("awp", [128, 2, 3], BF16, isOutput=False)
    owt = nc.declare_dram_parameter("owt", [128, 2, 256], BF16, isOutput=False)
    obr = nc.declare_dram_parameter("obr", [1, 256], BF16, isOutput=False)
    obc = nc.declare_dram_parameter("obc", [128, 2, 2], F32, isOutput=False)
    gwt = nc.declare_dram_parameter("gwt", [128, 2, 9, 256], BF16, isOutput=False)
    out = nc.declare_dram_parameter("out", [PB, C, H, W], F32, isOutput=True)

    with tile.TileContext(nc) as tc:
        _build_body(nc, tc, xb, dwp, awp, owt, obr, obc, gwt, out, u_vals, reps)
    nc.finalize()
    return nc


def _build_body(nc, tc, xb, dwp, awp, owt, obr, obc, gwt, out, u_vals, reps=1):
    from contextlib import ExitStack

    ctx = ExitStack()
    with ctx:
        singles = ctx.enter_context(tc.tile_pool(name="singles", bufs=1))
        stage = ctx.enter_context(tc.tile_pool(name="stage", bufs=2))
        accs0 = ctx.enter_context(tc.tile_pool(name="accs0", bufs=2))
        accs1 = ctx.enter_context(tc.tile_pool(name="accs1", bufs=1))
        prods = ctx.enter_context(tc.tile_pool(name="prods", bufs=5))
        a1p = ctx.enter_context(tc.tile_pool(name="a1p", bufs=1))
        smalls = ctx.enter_context(tc.tile_pool(name="smalls", bufs=2))
        outp = ctx.enter_context(tc.tile_pool(name="outp", bufs=2))
        tgp = ctx.enter_context(tc.tile_pool(name="tgp", bufs=16))
        dramp = ctx.enter_context(tc.tile_pool(name="dramp", bufs=2, space="DRAM"))

        # ---- constants / weights (resident) ----
        dwp_sb = singles.tile([128, 2, 30], F32)
        nc.sync.dma_start(out=dwp_sb, in_=dwp[:, :, :])
        awp_sb = singles.tile([128, 2, 3], BF16)
        nc.sync.dma_start(out=awp_sb, in_=awp[:, :, :])
        owt_sb = singles.tile([128, 2, 256], BF16)
        nc.sync.dma_start(out=owt_sb, in_=owt[:, :, :])
        obc_sb = singles.tile([128, 2, 2], F32)
        nc.sync.dma_start(out=obc_sb, in_=obc[:, :, :])
        gwt_sb = singles.tile([128, 2, 9, 256], BF16)
        nc.sync.dma_start(out=gwt_sb, in_=gwt[:, :, :, :])
        ident = singles.tile([128, 128], F32)
        make_identity(nc, ident)

        # persistent padded-x frames (even + element-shifted odd), borders
        # zeroed once; the per-image DMA rewrites only the interior
        xpe = []
        xpo = []
        for ck in range(2):
            e = singles.tile([128, WP, WP], BF16, tag=f"xpe{ck}")
            o = singles.tile([128, WP, WP], BF16, tag=f"xpo{ck}")
            for t_, c0, c1 in ((e, PAD, PAD + W), (o, PAD - 1, PAD - 1 + W)):
                nc.gpsimd.memset(t_[:, 0:PAD, :], 0.0)
                nc.gpsimd.memset(t_[:, PAD + H :, :], 0.0)
                nc.gpsimd.memset(t_[:, PAD : PAD + H, 0:c0], 0.0)
                nc.gpsimd.memset(t_[:, PAD : PAD + H, c1:], 0.0)
            xpe.append(e)
            xpo.append(o)

        # tiny "touch" reads concentrate cross-engine waits off the hot ops
        scr = singles.tile([128, 8], F32)
        scra = singles.tile([128, 8], F32)
        nc.vector.tensor_copy(scr[:, 0:1], dwp_sb[:, 0, 0:1])
        for i, t_ in enumerate(xpe + xpo):
            nc.vector.tensor_copy(scr[:, i + 1 : i + 2], t_[:, 0:1, 0])
        nc.scalar.copy(scra[:, 5:6], dwp_sb[:, 0, 0:1])
        nc.scalar.copy(scra[:, 6:7], xpe[0][:, 0:1, 0])

        def w_ap(ck, k, t):
            return dwp_sb[:, ck, k * 9 + t : k * 9 + t + 1]

        def b_ap(ck, k):
            return dwp_sb[:, ck, 27 + k : 27 + k + 1]

        def phase_a(img, ps_l, ps_t):
            # ---- attention: transposed logits [hw_tile(128), 3] ----
            xlg = []
            for ck in range(2):
                s = stage.tile([128, HW], BF16, tag="xlg")
                nc.scalar.dma_start(
                    out=s.rearrange("p (h w) -> p h w", h=H),
                    in_=xb[img, ck * 128 : (ck + 1) * 128, :, :],
                )
                xlg.append(s)
            lps = ps_l.tile([128, 96], F32)
            for j in range(32):
                for ck in range(2):
                    nc.tensor.matmul(
                        lps[:, 3 * j : 3 * j + 3],
                        lhsT=xlg[ck][:, 128 * j : 128 * (j + 1)],
                        rhs=awp_sb[:, ck, :],
                        start=(ck == 0),
                        stop=(ck == 1),
                    )
            esb = smalls.tile([128, 96], F32)
            nc.scalar.activation(esb, lps[:, :], ActFn.Exp)
            e3 = esb.rearrange("p (j k) -> p j k", k=3)
            ssum = smalls.tile([128, 32], F32)
            nc.vector.tensor_scalar(
                out=ssum, in0=e3[:, :, 0], scalar1=float(u_vals[0]), scalar2=None,
                op0=AluOp.mult,
            )
            for k in (1, 2):
                nc.vector.scalar_tensor_tensor(
                    out=ssum, in0=e3[:, :, k], scalar=float(u_vals[k]), in1=ssum,
                    op0=AluOp.mult, op1=AluOp.add,
                )
            rsum = smalls.tile([128, 32], F32)
            nc.vector.reciprocal_approx_fast(rsum, ssum)
            a1t = smalls.tile([128, 96], F32)
            a1t3 = a1t.rearrange("p (j k) -> p j k", k=3)
            for k in range(3):
                nc.vector.scalar_tensor_tensor(
                    out=a1t3[:, :, k], in0=e3[:, :, k], scalar=float(u_vals[k]),
                    in1=ssum, op0=AluOp.mult, op1=AluOp.add,
                )
                nc.vector.tensor_tensor(
                    out=a1t3[:, :, k], in0=a1t3[:, :, k], in1=rsum, op=AluOp.mult
                )
            # transpose [128, 96] -> [96, 128]; bounce via DRAM to broadcast
            tps = ps_t.tile([96, 128], F32)
            nc.tensor.transpose(tps[:, :], a1t[:, :], ident[:, :])
            a1rows = smalls.tile([96, 128], BF16)
            nc.vector.tensor_copy(a1rows, tps[:, :])
            a1dram = dramp.tile([96, 128], BF16)
            nc.sync.dma_start(out=a1dram, in_=a1rows)
            return a1dram

        def phase_b(img, a1dram, ps_g, ps_o):
            # ---- interior loads ----
            for ck in range(2):
                nc.sync.dma_start(
                    out=xpe[ck][:, PAD : PAD + H, PAD : PAD + W],
                    in_=xb[img, ck * 128 : (ck + 1) * 128, :, :],
                )
                nc.gpsimd.dma_start(
                    out=xpo[ck][:, PAD : PAD + H, PAD - 1 : PAD - 1 + W],
                    in_=xb[img, ck * 128 : (ck + 1) * 128, :, :],
                )
                nc.vector.tensor_copy(scr[:, 0:1], xpe[ck][:, PAD : PAD + 1, PAD])
                nc.vector.tensor_copy(scr[:, 1:2], xpo[ck][:, PAD : PAD + 1, PAD])
                nc.scalar.copy(scra[:, 0:1], xpe[ck][:, PAD : PAD + 1, PAD])
                nc.scalar.copy(scra[:, 1:2], xpo[ck][:, PAD : PAD + 1, PAD])
            a1sb = []
            for k in range(3):
                a1k = a1p.tile([128, 32, 128], BF16, tag=f"a1{k}")
                bsrc = bass.AP(
                    tensor=a1dram.tensor,
                    offset=a1dram.offset + k * 128,
                    ap=[[0, 128], [3 * 128, 32], [1, 128]],
                )
                nc.gpsimd.dma_start(out=a1k, in_=bsrc)
                a1sb.append(a1k)

            # ---- depthwise taps + G-branch matmuls, interleaved ----
            # Emit order matters: each engine executes its stream in order.
            # The d=2 branch's G matmuls (PE) run concurrently with the DVE
            # tap chains of the d=1/d=5 branches; their tg combines are
            # spliced between the four tap chains so the P_g PSUM slots
            # recycle during the tap phase.
            a1f = a1sb[1].rearrange("p j i -> p (j i)")

            def emit_g_mms(ok, n):
                pg = ps_g.tile([128, NBW, W], F32, tag="pg")
                first = True
                for ck in range(2):
                    for t in range(9):
                        dy, dx = t // 3 - 1, t % 3 - 1
                        r0 = PAD + dy * 2 + n * NBW
                        c0 = PAD + dx * 2
                        nc.tensor.matmul(
                            pg[:, :, :],
                            lhsT=gwt_sb[:, ck, t, ok * 128 : (ok + 1) * 128],
                            rhs=xpe[ck][:, r0 : r0 + NBW, c0 : c0 + W],
                            start=first,
                            stop=(ck == 1 and t == 8),
                        )
                        first = False
                return pg

            def emit_tg(ok, n, pg):
                tg = tgp.tile([128, NBW * W], BF16, tag="tg")
                nc.vector.scalar_tensor_tensor(
                    out=tg, in0=pg.rearrange("p a b -> p (a b)"),
                    scalar=obc_sb[:, ok, 1:2],
                    in1=a1f[:, n * NBW * W : (n + 1) * NBW * W],
                    op0=AluOp.add, op1=AluOp.mult,
                )
                return tg

            tiles = [(ok, n) for ok in range(2) for n in range(NB)]
            g_iter = iter(tiles)
            pg_map = {}
            tg_map = {}
            fused = []
            batch = 0
            for ck in range(2):
                acc = []
                for k, d in ((0, 1), (2, 5)):
                    apool = accs0 if ck == 0 else accs1
                    a = apool.tile([128, H, W], BF16, tag=f"acc{ck}{k}")
                    def tap_view(t):
                        dy, dx = t // 3 - 1, t % 3 - 1
                        r0, c0 = PAD + dy * d, PAD + dx * d
                        if c0 % 2 == 0:
                            return xpe[ck][:, r0 : r0 + H, c0 : c0 + W]
                        return xpo[ck][:, r0 : r0 + H, c0 - 1 : c0 - 1 + W]

                    def dve_prod(t):
                        p = prods.tile([128, H, W], BF16, tag="prod")
                        if t == 0:
                            nc.vector.tensor_scalar(
                                out=p, in0=tap_view(t), scalar1=w_ap(ck, k, t),
                                scalar2=b_ap(ck, k), op0=AluOp.mult,
                                op1=AluOp.add,
                            )
                        else:
                            nc.vector.tensor_scalar(
                                out=p, in0=tap_view(t), scalar1=w_ap(ck, k, t),
                                scalar2=None, op0=AluOp.mult,
                            )
                        return p

                    def act_prod(t):
                        p = prods.tile([128, H, W], BF16, tag="prod")
                        nc.scalar.activation(
                            p, tap_view(t), ActFn.Copy, bias=0.0,
                            scale=w_ap(ck, k, t),
                        )
                        return p

                    dve_ts = [t for t in range(9) if t not in ACT_TAPS]
                    # interleave adds with products so prod slots recycle
                    p0 = dve_prod(dve_ts[0])
                    pa1 = act_prod(ACT_TAPS[0])
                    p1 = dve_prod(dve_ts[1])
                    nc.vector.tensor_tensor(out=a, in0=p0, in1=p1, op=AluOp.add)
                    for t in dve_ts[2:]:
                        p = dve_prod(t)
                        nc.vector.tensor_tensor(out=a, in0=a, in1=p, op=AluOp.add)
                    nc.vector.tensor_tensor(out=a, in0=a, in1=pa1, op=AluOp.add)
                    for t in ACT_TAPS[1:]:
                        p = act_prod(t)
                        nc.vector.tensor_tensor(out=a, in0=a, in1=p, op=AluOp.add)
                    acc.append(a)
                    # splice in a batch of 4 tg combines
                    for _ in range(4):
                        ok_n = next(g_iter, None)
                        if ok_n is not None:
                            okx, nx = ok_n
                            pg = pg_map.pop(ok_n, None)
                            if pg is None:
                                pg = emit_g_mms(okx, nx)
                            tg_map[ok_n] = emit_tg(okx, nx, pg)
                # apply (1 + attn_k) in place
                af = [a.rearrange("p h w -> p (h w)") for a in acc]
                gf = [a1sb[k].rearrange("p j i -> p (j i)") for k in (0, 2)]
                for i in range(2):
                    nc.vector.tensor_tensor(
                        out=af[i], in0=af[i], in1=gf[i], op=AluOp.mult
                    )
                fused.append(acc)

            # ---- P_a matmuls + final combine per tile ----
            for ok, n in tiles:
                pso = ps_o.tile([128, NBW, W], F32)
                first = True
                for ck in range(2):
                    for i in range(2):
                        nc.tensor.matmul(
                            pso[:, :, :],
                            lhsT=owt_sb[:, ck, ok * 128 : (ok + 1) * 128],
                            rhs=fused[ck][i][:, n * NBW : (n + 1) * NBW, :],
                            start=first,
                            stop=(ck == 1 and i == 1),
                        )
                        first = False
                osb = outp.tile([128, NBW * W], F32)
                nc.vector.scalar_tensor_tensor(
                    out=osb, in0=pso.rearrange("p a b -> p (a b)"),
                    scalar=obc_sb[:, ok, 0:1], in1=tg_map[(ok, n)],
                    op0=AluOp.add, op1=AluOp.add,
                )
                nc.sync.dma_start(
                    out=out[
                        img,
                        ok * 128 : (ok + 1) * 128,
                        n * NBW : (n + 1) * NBW,
                        :,
                    ],
                    in_=osb.rearrange("p (a b) -> p a b", a=NBW),
                )

        for _ in range(reps):
            with tc.tile_pool(name="ps_l", bufs=2, space="PSUM") as ps_l, \
                 tc.tile_pool(name="ps_t", bufs=2, space="PSUM") as ps_t:
                a1drams = [phase_a(img, ps_l, ps_t) for img in range(PB)]
            with tc.tile_pool(name="ps_g", bufs=4, space="PSUM") as ps_g, \
                 tc.tile_pool(name="ps_o", bufs=4, space="PSUM") as ps_o:
                for img in range(PB):
                    phase_b(img, a1drams[img], ps_g, ps_o)


def make_in_maps(x, dw_w, dw_b, attn_w, attn_b, out_w, out_b):
    """Host-side packing. Returns (in_maps list for 8 cores, u_vals)."""
    import ml_dtypes

    bf16 = ml_dtypes.bfloat16
    xb = np.ascontiguousarray(x).astype(bf16)  # [16,256,64,64]

    # dw weights: [2(ck), 128, 30] f32: cols 0..26 = branch k tap t (k*9+t),
    # cols 27..29 = per-branch bias
    dwp = np.zeros((128, 2, 30), np.float32)
    w = dw_w.reshape(3, C, 9)  # [k, c, t]
    for ck in range(2):
        cs = slice(ck * 128, (ck + 1) * 128)
        dwp[:, ck, :27] = (
            w[:, cs, :].transpose(1, 0, 2).reshape(128, 27)
        )  # [c, k*9+t]
        dwp[:, ck, 27:] = dw_b[:, cs].T  # [c, k]
    awp = np.ascontiguousarray(
        np.stack([attn_w.T[:128], attn_w.T[128:]], axis=1)
    ).astype(bf16)  # [128(c), 2, 3]
    owt = np.ascontiguousarray(
        np.stack([out_w.T[:128], out_w.T[128:]], axis=1)
    ).astype(bf16)  # [128(c), 2, 256(o)]
    obr = np.ascontiguousarray(out_b[None, :]).astype(bf16)  # [1, 256]
    q = out_w @ dw_b[1]  # [256] = sum_c out_w[o,c]*dw_b[1,c]
    obc = np.zeros((128, 2, 2), np.float32)
    obc[:, :, 0] = out_b.reshape(2, 128).T
    obc[:, :, 1] = q.reshape(2, 128).T
    # gwt[c, ck, t, o] = out_w[o, ck*128+c] * dw_w[1, ck*128+c, 0, t//3, t%3]
    w1 = dw_w[1].reshape(C, 9)  # [c, t]
    gwt_full = out_w.T[:, None, :] * w1[:, :, None]  # [c, t, o]
    gwt = np.ascontiguousarray(
        np.stack([gwt_full[:128], gwt_full[128:]], axis=1)
    ).astype(bf16)  # [128, 2, 9, 256]
    u_vals = np.exp(attn_b.astype(np.float64)).astype(np.float32)

    in_maps = []
    for i in range(N_CORES):
        in_maps.append(
            {
                "xb": np.ascontiguousarray(xb[i * PB : (i + 1) * PB]),
                "dwp": dwp,
                "awp": awp,
                "owt": owt,
                "obr": obr,
                "obc": obc,
                "gwt": gwt,
            }
        )
    return in_maps, u_vals


def kernel(**inputs) -> np.ndarray:
    x = np.asarray(inputs["x"], np.float32)
    in_maps, u_vals = make_in_maps(
        x,
        np.asarray(inputs["dw_w"], np.float32),
        np.asarray(inputs["dw_b"], np.float32),
        np.asarray(inputs["attn_w"], np.float32),
        np.asarray(inputs["attn_b"], np.float32),
        np.asarray(inputs["out_w"], np.float32),
        np.asarray(inputs["out_b"], np.float32),
    )
    nc = build_bass(u_vals)

    from concourse.bass_utils import run_bass_kernel_spmd

    res = run_bass_kernel_spmd(nc, in_maps, core_ids=list(range(N_CORES)))
    outs = [res.results[i]["out"] for i in range(N_CORES)]
    return np.concatenate(outs, axis=0).astype(np.float32)


if __name__ == "__main__":
    # smoke: build only
    nc = build_bass([1.0, 1.0, 1.0])
    print("built ok")

